# revision 11
# baseline (speedup 1.0000x reference)
"""Causal self-attention (B=2, T=2048, E=1024, H=16) on 8 trn2 NeuronCores.

Sharding: core = b*4 + g  (b = batch index, g = head-group of 4 heads).
Each core computes its 4 heads' attention for its batch plus a partial
output projection; the host sums the 4 partials per batch.

Device layout ("half-major"): qT/kT each live as 2 SBUF tiles per
q-window; tile s holds RoPE half s of all 4 local heads:
    partitions [32h : 32h+32) = head h dims [32s : 32s+32)
RoPE is 6 wide DVE ops per (tensor, window).  The QK^T matmul for head h
is a pair of K=32 matmuls (one per half) at the SAME PE row-group 32h
accumulating into the same psum window (legal), while the 4 heads write
4 distinct psum banks from 4 distinct row-groups (concurrent execution).
Scores are computed transposed (S^T = K Q^T, k on partitions) so softmax
exp feeds the attention matmul with no transposes; V carries an appended
ones column so row 64 of the attention psum is the softmax denominator.
All matmul operands are bf16 (fp32 is 1/4 rate on PE); fp32 accumulation.
"""

import numpy as np
import ml_dtypes

BF16 = ml_dtypes.bfloat16

B, T, E = 2, 2048, 1024
H, HD = 16, 64
G = 4             # head groups (cores per batch)
HL = H // G       # heads per core
DL = HL * HD      # local qkv dim = 256
TC = 512          # T chunk (matmul moving free dim)
NJ = T // TC      # 4 q-windows
KC = 128          # k-chunk (contraction tile for attention)
NC_ = T // KC     # 16 k-chunks
SCALE = 1.0 / float(np.sqrt(HD))

_CACHE = {}


def _build_bass():
    import concourse.mybir as mybir
    import concourse.tile as tile
    from concourse import bacc

    f32 = mybir.dt.float32
    bf16 = mybir.dt.bfloat16
    EXP = mybir.ActivationFunctionType.Exp

    nc = bacc.Bacc("TRN2", target_bir_lowering=False, debug=False)
    xt_d = nc.dram_tensor("xt", [E, T], bf16, kind="ExternalInput").ap()
    w_d = nc.dram_tensor("w", [E, 3 * DL], bf16, kind="ExternalInput").ap()
    wo_d = nc.dram_tensor("wo", [DL, E], bf16, kind="ExternalInput").ap()
    cos_d = [nc.dram_tensor(f"cos{s}", [128, T], bf16, kind="ExternalInput").ap()
             for s in range(2)]
    sin_d = [nc.dram_tensor(f"sin{s}", [128, T], bf16, kind="ExternalInput").ap()
             for s in range(2)]
    y_d = nc.dram_tensor("y", [T, E], bf16, kind="ExternalOutput").ap()

    NKK = E // KC  # 8 contraction chunks for the projections

    with tile.TileContext(nc) as tc:
        with (
            tc.tile_pool(name="consts", bufs=1) as consts,
            tc.tile_pool(name="stp", bufs=2, space="PSUM") as stp,
            tc.tile_pool(name="avp", bufs=1, space="PSUM") as avp,
            tc.tile_pool(name="tmp_sb", bufs=6) as tmp_sb,
            tc.tile_pool(name="est_sb", bufs=3) as est_sb,
            tc.tile_pool(name="attn_sb", bufs=1) as attn_sb,
            tc.tile_pool(name="ysb_p", bufs=2) as ysb_p,
            tc.tile_pool(name="small_sb", bufs=3) as small_sb,
        ):
            # ---- constants ----
            xt, w = [], []
            for i in range(NKK):
                tw = consts.tile([KC, 3 * DL], bf16, tag=f"w{i}", name=f"w{i}")
                nc.sync.dma_start(out=tw, in_=w_d[i * KC:(i + 1) * KC, :])
                w.append(tw)
                t = consts.tile([KC, T], bf16, tag=f"xt{i}", name=f"xt{i}")
                nc.sync.dma_start(out=t, in_=xt_d[i * KC:(i + 1) * KC, :])
                xt.append(t)
            wo = []
            for tau in range(2):
                t = consts.tile([128, E], bf16, tag=f"wo{tau}")
                nc.sync.dma_start(out=t, in_=wo_d[tau * 128:(tau + 1) * 128, :])
                wo.append(t)
            cosx, sinx = [], []
            for s in range(2):
                t = consts.tile([128, T], bf16, tag=f"cos{s}", name=f"cosx{s}")
                nc.sync.dma_start(out=t, in_=cos_d[s])
                cosx.append(t)
                t = consts.tile([128, T], bf16, tag=f"sin{s}", name=f"sinx{s}")
                nc.sync.dma_start(out=t, in_=sin_d[s])
                sinx.append(t)

            # triangular band mask: band[p, f] = 1 if f >= p else 0
            band = consts.tile([128, KC], bf16, tag="band")
            nc.gpsimd.memset(band, 1.0)
            nc.gpsimd.affine_select(
                out=band, in_=band, compare_op=mybir.AluOpType.is_ge, fill=0.0,
                base=0, pattern=[[1, KC]], channel_multiplier=-1,
            )

            # resident projection outputs, indexed [s][j]
            qsb = [[consts.tile([128, TC], bf16, tag=f"q{s}_{j}", name=f"q{s}_{j}")
                    for j in range(NJ)] for s in range(2)]
            ksb = [[consts.tile([128, TC], bf16, tag=f"k{s}_{j}", name=f"k{s}_{j}")
                    for j in range(NJ)] for s in range(2)]
            vsb = [consts.tile([128, HL * 65], bf16, tag=f"v{c}", name=f"v{c}")
                   for c in range(NC_)]
            # natural-contiguous copies: tile tau rows [64l:64l+64) = head
            # 2tau+l dims 0..63
            qn = [[consts.tile([128, TC], bf16, tag=f"qn{tau}_{j}",
                               name=f"qn{tau}_{j}") for j in range(NJ)]
                  for tau in range(2)]
            kn = [[consts.tile([128, TC], bf16, tag=f"kn{tau}_{j}",
                               name=f"kn{tau}_{j}") for j in range(NJ)]
                  for tau in range(2)]

            def emit_pack(dst, half, jj):
                # dst[tau][jj][64l+32s : 64l+32s+32] = half[s][jj][32(2tau+l):+32]
                for tau in range(2):
                    for ll in range(2):
                        for s in range(2):
                            nc.gpsimd.tensor_copy(
                                dst[tau][jj][64 * ll + 32 * s:
                                             64 * ll + 32 * s + 32, :],
                                half[s][jj][32 * (2 * tau + ll):
                                            32 * (2 * tau + ll) + 32, :])

            def emit_proj_qk(jj, m0, dst):
                """Project m-chunks (m0, m0+1) for window jj and apply RoPE
                into dst[0][jj] / dst[1][jj]."""
                js = slice(jj * TC, (jj + 1) * TC)
                ps = stp.tile([128, 2 * TC], f32, tag="st", name=f"pqk{m0}_{jj}")
                for half in range(2):
                    for kk in range(NKK):
                        nc.tensor.matmul(
                            ps[:, half * TC:(half + 1) * TC],
                            lhsT=w[kk][:, (m0 + half) * 128:(m0 + half + 1) * 128],
                            rhs=xt[kk][:, js],
                            start=(kk == 0), stop=(kk == NKK - 1))
                p0, p1 = ps[:, 0:TC], ps[:, TC:2 * TC]
                ta = tmp_sb.tile([128, TC], f32, tag="ropeA", name=f"ra{m0}_{jj}")
                tb = tmp_sb.tile([128, TC], f32, tag="ropeB", name=f"rb{m0}_{jj}")
                nc.vector.tensor_mul(ta, p0, cosx[0][:, js])
                nc.vector.tensor_mul(tb, p1, sinx[0][:, js])
                nc.vector.tensor_sub(dst[0][jj], ta, tb)
                nc.vector.tensor_mul(ta, p1, cosx[1][:, js])
                nc.vector.tensor_mul(tb, p0, sinx[1][:, js])
                nc.vector.tensor_add(dst[1][jj], ta, tb)

            def emit_proj_v(jj):
                js0 = jj * (TC // KC)
                ps = stp.tile([128, 4 * DL], f32, tag="st", name=f"pv{jj}")
                for tt in range(TC // KC):
                    c = js0 + tt
                    for kk in range(NKK):
                        nc.tensor.matmul(
                            ps[:, tt * DL:(tt + 1) * DL],
                            lhsT=xt[kk][:, c * KC:(c + 1) * KC],
                            rhs=w[kk][:, 2 * DL:3 * DL],
                            start=(kk == 0), stop=(kk == NKK - 1))
                for tt in range(TC // KC):
                    c = js0 + tt
                    vv = vsb[c].rearrange("p (h d) -> p h d", h=HL)
                    nc.gpsimd.memset(vv[:, :, 64:65], 1.0)
                    nc.vector.tensor_copy(
                        vv[:, :, 0:64],
                        ps[:, tt * DL:(tt + 1) * DL].rearrange(
                            "p (h d) -> p h d", h=HL))

            def emit_proj(jj, part):
                if part == 0:
                    emit_proj_qk(jj, 0, qsb)
                    emit_pack(qn, qsb, jj)
                elif part == 1:
                    emit_proj_qk(jj, 2, ksb)
                    emit_pack(kn, ksb, jj)
                else:
                    emit_proj_v(jj)

            # prologue: projections for window 0
            for part in range(3):
                emit_proj(0, part)
            ats = []

            for j in range(NJ):
                nch = 4 * (j + 1)          # causal k-chunks for this window
                # interleave next window's projection emission into the
                # attention chunk loop (keeps PE busy while ACT does exp)
                proj_at = {}
                if j + 1 < NJ:
                    for part in range(3):
                        proj_at[max(0, (nch * (part + 1)) // 4 - 1)] = part

                at = [attn_sb.tile([128, TC], bf16, tag=f"attn{tau}_{j}",
                                   name=f"attn{tau}_{j}") for tau in range(2)]
                ats.append(at)
                av4 = avp.tile([128, 4 * TC], f32, tag="av", name=f"av_{j}")
                for c in range(nch):
                    d = c - 4 * j          # 0..3 on the diagonal
                    sts = [stp.tile([128, 2 * TC], f32, tag="st",
                                    name=f"st{j}_{c}_{i}") for i in range(2)]
                    # single K=64 matmul per head; the two heads of a
                    # parity class sit on disjoint row-groups
                    for ll in range(2):
                        for tau in range(2):
                            h = 2 * tau + ll
                            stt = sts[h // 2]
                            wnd = slice((h % 2) * TC, (h % 2 + 1) * TC)
                            nc.tensor.matmul(
                                stt[:, wnd],
                                lhsT=kn[tau][c // 4][
                                    64 * ll:64 * ll + 64,
                                    (c % 4) * KC:(c % 4 + 1) * KC],
                                rhs=qn[tau][j][64 * ll:64 * ll + 64, :],
                                start=True, stop=True,
                                tile_position=(64 * ll, 0))
                    ests = []
                    ecoff = KC * d if d > 0 else 0
                    for i in range(2):
                        est = est_sb.tile([128, 2 * TC], bf16, tag="est",
                                          name=f"est{j}_{c}_{i}")
                        if ecoff:
                            nc.scalar.activation(
                                est.rearrange("p (w c) -> p w c", w=2)[
                                    :, :, ecoff:],
                                sts[i].rearrange("p (w c) -> p w c", w=2)[
                                    :, :, ecoff:],
                                EXP, scale=SCALE)
                        else:
                            nc.scalar.activation(est, sts[i], EXP, scale=SCALE)
                        ests.append(est)
                    if d >= 0:
                        for h in range(HL):
                            bs = slice((h % 2) * TC + KC * d,
                                       (h % 2) * TC + KC * (d + 1))
                            nc.vector.tensor_mul(
                                ests[h // 2][:, bs], ests[h // 2][:, bs], band)
                    coff = KC * d if d > 0 else 0
                    for h in range(HL):
                        nc.tensor.matmul(
                            av4[0:65, h * TC + coff:(h + 1) * TC],
                            lhsT=vsb[c][:, 65 * h:65 * h + 65],
                            rhs=ests[h // 2][:, (h % 2) * TC + coff:
                                             (h % 2 + 1) * TC],
                            start=(c == 0), stop=(c == nch - 1))
                    if c in proj_at:
                        emit_proj(j + 1, proj_at[c])

                # softmax denominator divide -> bf16 attn tiles
                for h in range(HL):
                    hw = slice(h * TC, (h + 1) * TC)
                    dn = small_sb.tile([1, TC], f32, tag="denom",
                                       name=f"dn{j}_{h}")
                    nc.vector.tensor_copy(dn, av4[64:65, hw])
                    rc = small_sb.tile([1, TC], f32, tag="recip",
                                       name=f"rc{j}_{h}")
                    nc.vector.reciprocal_approx_fast(out=rc, in_=dn)
                    rb = small_sb.tile([64, TC], f32, tag="rbcast",
                                       name=f"rb{j}_{h}")
                    nc.gpsimd.partition_broadcast(rb, rc)
                    nc.vector.tensor_mul(
                        at[h // 2][64 * (h % 2):64 * (h % 2) + 64, :],
                        av4[0:64, hw], rb)

            # ---- epilogue: output projection for all windows ----
            for j in range(NJ):
                at = ats[j]
                for n in range(2):
                    for th in range(2):
                        yp = stp.tile([128, 2 * TC], f32, tag="st",
                                      name=f"yp{j}_{n}_{th}")
                        for ti in range(2):
                            tt = 2 * th + ti
                            for tau in range(2):
                                nc.tensor.matmul(
                                    yp[:, ti * TC:(ti + 1) * TC],
                                    lhsT=at[tau][:, tt * KC:(tt + 1) * KC],
                                    rhs=wo[tau][:, n * TC:(n + 1) * TC],
                                    start=(tau == 0), stop=(tau == 1))
                        ys = ysb_p.tile([128, 2 * TC], bf16, tag="y",
                                        name=f"ys{j}_{n}_{th}")
                        nc.vector.tensor_copy(ys, yp)
                        nc.sync.dma_start(
                            out=y_d[j * TC + th * 2 * KC:
                                    j * TC + (th + 1) * 2 * KC,
                                    n * TC:(n + 1) * TC].rearrange(
                                        "(a p) c -> p a c", p=128),
                            in_=ys.rearrange("p (a c) -> p a c", a=2))

    nc.compile()
    return nc


def _host_inputs(x, cos, sin, w_qkv, w_out):
    """Shard + lay out the full inputs for the 8 cores."""
    # half-major tables: row 32h+i of table s = cos/sin[t, 32s+i]
    cosx = [np.ascontiguousarray(
        np.tile(cos[:, 32 * s:32 * (s + 1)].T, (4, 1))).astype(BF16)
        for s in range(2)]
    sinx = [np.ascontiguousarray(
        np.tile(sin[:, 32 * s:32 * (s + 1)].T, (4, 1))).astype(BF16)
        for s in range(2)]

    xts = [np.ascontiguousarray(x[b].T).astype(BF16) for b in range(B)]

    in_maps = []
    for core in range(8):
        b, g = divmod(core, G)
        # qkv column permutation: m-chunk (blk, s) = half s of all 4 heads
        cols = []
        for blk in range(2):               # 0 = q, 1 = k
            for s in range(2):
                for h in range(HL):
                    base = blk * E + (G * g + h) * HD + 32 * s
                    cols.extend(range(base, base + 32))
        cols.extend(range(2 * E + DL * g, 2 * E + DL * (g + 1)))   # v natural
        wl = np.ascontiguousarray(w_qkv[:, cols]).astype(BF16)     # (E, 768)
        wol = np.ascontiguousarray(w_out[DL * g:DL * (g + 1), :]).astype(BF16)
        in_maps.append({
            "xt": xts[b], "w": wl, "wo": wol,
            "cos0": cosx[0], "cos1": cosx[1],
            "sin0": sinx[0], "sin1": sinx[1],
        })
    return in_maps


def kernel(x, cos, sin, w_qkv, w_out):
    from concourse import bass_utils

    if "nc" not in _CACHE:
        _CACHE["nc"] = _build_bass()
    nc = _CACHE["nc"]

    in_maps = _host_inputs(
        np.asarray(x, dtype=np.float32), np.asarray(cos, dtype=np.float32),
        np.asarray(sin, dtype=np.float32), np.asarray(w_qkv, dtype=np.float32),
        np.asarray(w_out, dtype=np.float32))

    res = bass_utils.run_bass_kernel_spmd(nc, in_maps, core_ids=list(range(8)))

    y = np.zeros((B, T, E), dtype=np.float32)
    for core in range(8):
        b = core // G
        y[b] += res.results[core]["y"].astype(np.float32)
    return y


# revision 14
# speedup vs baseline: 1.1803x; 1.1803x over previous
"""Causal self-attention (B=2, T=2048, E=1024, H=16) on 8 trn2 NeuronCores.

Sharding: core = b*4 + g  (b = batch index, g = head-group of 4 heads).
Each core computes its 4 heads' attention for its batch plus a partial
output projection; the host sums the 4 partials per batch.

RoPE trick: rotate_half(q) = q @ R is linear, so the host precomputes
W_rot = W @ R and the kernel projects twice; q' = (x@W)*cos + (x@W_rot)*sin
is then 3 wide DVE ops in the natural head-contiguous layout:
  qT tile tau rows [64l : 64l+64) = head 2tau+l dims 0..63, T on free.
Scores are computed transposed (S^T = K Q^T, k on partitions, K=64
contraction) so softmax exp feeds the attention matmul with no
transposes; the two heads of a tile sit on disjoint PE row-groups and
psum banks so their score matmuls run concurrently.  V carries an
appended ones column so row 64 of the attention psum is the softmax
denominator.  All matmul operands bf16 (fp32 is 1/4 rate on PE); fp32
accumulation; exp skips fully-invalid diagonal columns.
"""

import numpy as np
import ml_dtypes

BF16 = ml_dtypes.bfloat16

B, T, E = 2, 2048, 1024
H, HD = 16, 64
G = 4             # head groups (cores per batch)
HL = H // G       # heads per core
DL = HL * HD      # local qkv dim = 256
TC = 512          # T chunk (matmul moving free dim)
NJ = T // TC      # 4 q-windows
KC = 128          # k-chunk (contraction tile for attention)
NC_ = T // KC     # 16 k-chunks
SCALE = 1.0 / float(np.sqrt(HD))

_CACHE = {}


def _build_bass():
    import concourse.mybir as mybir
    import concourse.tile as tile
    from concourse import bacc

    f32 = mybir.dt.float32
    bf16 = mybir.dt.bfloat16
    EXP = mybir.ActivationFunctionType.Exp

    nc = bacc.Bacc("TRN2", target_bir_lowering=False, debug=False)
    xt_d = nc.dram_tensor("xt", [E, T], bf16, kind="ExternalInput").ap()
    w_d = nc.dram_tensor("w", [E, 5 * DL], bf16, kind="ExternalInput").ap()
    wo_d = nc.dram_tensor("wo", [DL, E], bf16, kind="ExternalInput").ap()
    cos_d = nc.dram_tensor("cosf", [128, T], bf16, kind="ExternalInput").ap()
    sin_d = nc.dram_tensor("sinf", [128, T], bf16, kind="ExternalInput").ap()
    y_d = nc.dram_tensor("y", [T, E], bf16, kind="ExternalOutput").ap()

    NKK = E // KC  # 8 contraction chunks for the projections

    with tile.TileContext(nc) as tc:
        with (
            tc.tile_pool(name="consts", bufs=1) as consts,
            tc.tile_pool(name="stp", bufs=2, space="PSUM") as stp,
            tc.tile_pool(name="avp", bufs=1, space="PSUM") as avp,
            tc.tile_pool(name="tmp_sb", bufs=6) as tmp_sb,
            tc.tile_pool(name="est_sb", bufs=3) as est_sb,
            tc.tile_pool(name="attn_sb", bufs=1) as attn_sb,
            tc.tile_pool(name="ysb_p", bufs=2) as ysb_p,
            tc.tile_pool(name="small_sb", bufs=3) as small_sb,
        ):
            # ---- constants ----
            xt, w = [], []
            for i in range(NKK):
                tw = consts.tile([KC, 5 * DL], bf16, tag=f"w{i}", name=f"w{i}")
                nc.sync.dma_start(out=tw, in_=w_d[i * KC:(i + 1) * KC, :])
                w.append(tw)
                t = consts.tile([KC, T], bf16, tag=f"xt{i}", name=f"xt{i}")
                nc.sync.dma_start(out=t, in_=xt_d[i * KC:(i + 1) * KC, :])
                xt.append(t)
            wo = []
            for tau in range(2):
                t = consts.tile([128, E], bf16, tag=f"wo{tau}", name=f"wo{tau}")
                nc.sync.dma_start(out=t, in_=wo_d[tau * 128:(tau + 1) * 128, :])
                wo.append(t)
            cosf = consts.tile([128, T], bf16, tag="cosf")
            nc.sync.dma_start(out=cosf, in_=cos_d)
            sinf = consts.tile([128, T], bf16, tag="sinf")
            nc.sync.dma_start(out=sinf, in_=sin_d)

            # triangular band mask: band[p, f] = 1 if f >= p else 0
            band = consts.tile([128, KC], bf16, tag="band")
            nc.gpsimd.memset(band, 1.0)
            nc.gpsimd.affine_select(
                out=band, in_=band, compare_op=mybir.AluOpType.is_ge, fill=0.0,
                base=0, pattern=[[1, KC]], channel_multiplier=-1,
            )

            # resident projection outputs (natural head-contiguous layout)
            qn = [[consts.tile([128, TC], bf16, tag=f"qn{tau}_{j}",
                               name=f"qn{tau}_{j}") for j in range(NJ)]
                  for tau in range(2)]
            kn = [[consts.tile([128, TC], bf16, tag=f"kn{tau}_{j}",
                               name=f"kn{tau}_{j}") for j in range(NJ)]
                  for tau in range(2)]
            vsb = [consts.tile([128, HL * 65], bf16, tag=f"v{c}", name=f"v{c}")
                   for c in range(NC_)]

            def emit_proj_qk(jj, base, tau, dst):
                """Project [plain | rotated] m-chunks for head pair tau of
                window jj, then RoPE-combine into dst[tau][jj]."""
                js = slice(jj * TC, (jj + 1) * TC)
                ps = stp.tile([128, 2 * TC], f32, tag="st",
                              name=f"pqk{base}_{tau}_{jj}")
                for r in range(2):
                    cc = base + DL * r + 128 * tau
                    for kk in range(NKK):
                        nc.tensor.matmul(
                            ps[:, r * TC:(r + 1) * TC],
                            lhsT=w[kk][:, cc:cc + 128],
                            rhs=xt[kk][:, js],
                            start=(kk == 0), stop=(kk == NKK - 1))
                ta = tmp_sb.tile([128, TC], f32, tag="ropeA",
                                 name=f"ra{base}_{tau}_{jj}")
                tb = tmp_sb.tile([128, TC], f32, tag="ropeB",
                                 name=f"rb{base}_{tau}_{jj}")
                nc.vector.tensor_mul(ta, ps[:, 0:TC], cosf[:, js])
                nc.vector.tensor_mul(tb, ps[:, TC:2 * TC], sinf[:, js])
                nc.vector.tensor_add(dst[tau][jj], ta, tb)

            def emit_proj_v(jj):
                js0 = jj * (TC // KC)
                ps = stp.tile([128, 4 * DL], f32, tag="st", name=f"pv{jj}")
                for tt in range(TC // KC):
                    c = js0 + tt
                    for kk in range(NKK):
                        nc.tensor.matmul(
                            ps[:, tt * DL:(tt + 1) * DL],
                            lhsT=xt[kk][:, c * KC:(c + 1) * KC],
                            rhs=w[kk][:, 4 * DL:5 * DL],
                            start=(kk == 0), stop=(kk == NKK - 1))
                for tt in range(TC // KC):
                    c = js0 + tt
                    vv = vsb[c].rearrange("p (h d) -> p h d", h=HL)
                    nc.gpsimd.memset(vv[:, :, 64:65], 1.0)
                    nc.vector.tensor_copy(
                        vv[:, :, 0:64],
                        ps[:, tt * DL:(tt + 1) * DL].rearrange(
                            "p (h d) -> p h d", h=HL))

            def emit_proj(jj, part):
                if part == 0:
                    emit_proj_qk(jj, 0, 0, qn)
                elif part == 1:
                    emit_proj_qk(jj, 0, 1, qn)
                elif part == 2:
                    emit_proj_qk(jj, 2 * DL, 0, kn)
                elif part == 3:
                    emit_proj_qk(jj, 2 * DL, 1, kn)
                else:
                    emit_proj_v(jj)

            # prologue: projections for window 0
            for part in range(5):
                emit_proj(0, part)
            ats = []

            for j in range(NJ):
                nch = 4 * (j + 1)          # causal k-chunks for this window
                # interleave next window's projection emission into the
                # attention chunk loop (keeps PE busy while ACT does exp)
                proj_at = {}
                if j + 1 < NJ:
                    for part in range(5):
                        cpos = max(0, (nch * (part + 1)) // 6 - 1)
                        proj_at.setdefault(cpos, []).append(part)

                at = [attn_sb.tile([128, TC], bf16, tag=f"attn{tau}_{j}",
                                   name=f"attn{tau}_{j}") for tau in range(2)]
                ats.append(at)
                av4 = avp.tile([128, 4 * TC], f32, tag="av", name=f"av_{j}")
                for c in range(nch):
                    d = c - 4 * j          # 0..3 on the diagonal
                    sts = [stp.tile([128, 2 * TC], f32, tag="st",
                                    name=f"st{j}_{c}_{i}") for i in range(2)]
                    # one K=64 matmul per head; the two heads of a tile sit
                    # on disjoint row-groups and psum banks
                    for ll in range(2):
                        for tau in range(2):
                            h = 2 * tau + ll
                            stt = sts[h // 2]
                            wnd = slice((h % 2) * TC, (h % 2 + 1) * TC)
                            nc.tensor.matmul(
                                stt[:, wnd],
                                lhsT=kn[tau][c // 4][
                                    64 * ll:64 * ll + 64,
                                    (c % 4) * KC:(c % 4 + 1) * KC],
                                rhs=qn[tau][j][64 * ll:64 * ll + 64, :],
                                start=True, stop=True,
                                tile_position=(64 * ll, 0))
                    ests = []
                    ecoff = KC * d if d > 0 else 0
                    for i in range(2):
                        est = est_sb.tile([128, 2 * TC], bf16, tag="est",
                                          name=f"est{j}_{c}_{i}")
                        if ecoff:
                            nc.scalar.activation(
                                est.rearrange("p (w c) -> p w c", w=2)[
                                    :, :, ecoff:],
                                sts[i].rearrange("p (w c) -> p w c", w=2)[
                                    :, :, ecoff:],
                                EXP, scale=SCALE)
                        else:
                            nc.scalar.activation(est, sts[i], EXP, scale=SCALE)
                        ests.append(est)
                    if d >= 0:
                        for h in range(HL):
                            bs = slice((h % 2) * TC + KC * d,
                                       (h % 2) * TC + KC * (d + 1))
                            nc.vector.tensor_mul(
                                ests[h // 2][:, bs], ests[h // 2][:, bs], band)
                    coff = KC * d if d > 0 else 0
                    for h in range(HL):
                        nc.tensor.matmul(
                            av4[0:65, h * TC + coff:(h + 1) * TC],
                            lhsT=vsb[c][:, 65 * h:65 * h + 65],
                            rhs=ests[h // 2][:, (h % 2) * TC + coff:
                                             (h % 2 + 1) * TC],
                            start=(c == 0), stop=(c == nch - 1))
                    for part in proj_at.get(c, ()):
                        emit_proj(j + 1, part)

                # softmax denominator divide -> bf16 attn tiles
                for h in range(HL):
                    hw = slice(h * TC, (h + 1) * TC)
                    dn = small_sb.tile([1, TC], f32, tag="denom",
                                       name=f"dn{j}_{h}")
                    nc.vector.tensor_copy(dn, av4[64:65, hw])
                    rc = small_sb.tile([1, TC], f32, tag="recip",
                                       name=f"rc{j}_{h}")
                    nc.vector.reciprocal_approx_fast(out=rc, in_=dn)
                    rb = small_sb.tile([64, TC], f32, tag="rbcast",
                                       name=f"rb{j}_{h}")
                    nc.gpsimd.partition_broadcast(rb, rc)
                    nc.vector.tensor_mul(
                        at[h // 2][64 * (h % 2):64 * (h % 2) + 64, :],
                        av4[0:64, hw], rb)

            # ---- epilogue: output projection for all windows ----
            for j in range(NJ):
                at = ats[j]
                for n in range(2):
                    for th in range(2):
                        yp = stp.tile([128, 2 * TC], f32, tag="st",
                                      name=f"yp{j}_{n}_{th}")
                        for ti in range(2):
                            tt = 2 * th + ti
                            for tau in range(2):
                                nc.tensor.matmul(
                                    yp[:, ti * TC:(ti + 1) * TC],
                                    lhsT=at[tau][:, tt * KC:(tt + 1) * KC],
                                    rhs=wo[tau][:, n * TC:(n + 1) * TC],
                                    start=(tau == 0), stop=(tau == 1))
                        ys = ysb_p.tile([128, 2 * TC], bf16, tag="y",
                                        name=f"ys{j}_{n}_{th}")
                        nc.vector.tensor_copy(ys, yp)
                        nc.sync.dma_start(
                            out=y_d[j * TC + th * 2 * KC:
                                    j * TC + (th + 1) * 2 * KC,
                                    n * TC:(n + 1) * TC].rearrange(
                                        "(a p) c -> p a c", p=128),
                            in_=ys.rearrange("p (a c) -> p a c", a=2))

    nc.compile()
    return nc


def _host_inputs(x, cos, sin, w_qkv, w_out):
    """Shard + lay out the full inputs for the 8 cores."""
    # natural-layout tables: row 64*l + d = cos/sin[t, d]
    cosf = np.ascontiguousarray(np.tile(cos.T, (2, 1))).astype(BF16)
    sinf = np.ascontiguousarray(np.tile(sin.T, (2, 1))).astype(BF16)

    xts = [np.ascontiguousarray(x[b].T).astype(BF16) for b in range(B)]

    # rotate_half as a column transform: per head, W_rot[:, d] = -W[:, d+32]
    # for d<32 and W_rot[:, 32+d] = W[:, d]
    def rot_cols(wblk):
        out = np.empty_like(wblk)
        for h in range(HL):
            hs = wblk[:, h * HD:(h + 1) * HD]
            o = out[:, h * HD:(h + 1) * HD]
            o[:, 0:32] = -hs[:, 32:64]
            o[:, 32:64] = hs[:, 0:32]
        return out

    in_maps = []
    for core in range(8):
        b, g = divmod(core, G)
        qblk = w_qkv[:, G * g * HD:(G * g + HL) * HD]
        kblk = w_qkv[:, E + G * g * HD:E + (G * g + HL) * HD]
        vblk = w_qkv[:, 2 * E + DL * g:2 * E + DL * (g + 1)]
        wl = np.concatenate(
            [qblk, rot_cols(qblk), kblk, rot_cols(kblk), vblk],
            axis=1).astype(BF16)                                   # (E, 1280)
        wol = np.ascontiguousarray(w_out[DL * g:DL * (g + 1), :]).astype(BF16)
        in_maps.append({
            "xt": xts[b], "w": wl, "wo": wol, "cosf": cosf, "sinf": sinf,
        })
    return in_maps


def kernel(x, cos, sin, w_qkv, w_out):
    from concourse import bass_utils

    if "nc" not in _CACHE:
        _CACHE["nc"] = _build_bass()
    nc = _CACHE["nc"]

    in_maps = _host_inputs(
        np.asarray(x, dtype=np.float32), np.asarray(cos, dtype=np.float32),
        np.asarray(sin, dtype=np.float32), np.asarray(w_qkv, dtype=np.float32),
        np.asarray(w_out, dtype=np.float32))

    res = bass_utils.run_bass_kernel_spmd(nc, in_maps, core_ids=list(range(8)))

    y = np.zeros((B, T, E), dtype=np.float32)
    for core in range(8):
        b = core // G
        y[b] += res.results[core]["y"].astype(np.float32)
    return y


# revision 19
# speedup vs baseline: 1.2213x; 1.0348x over previous
"""Causal self-attention (B=2, T=2048, E=1024, H=16) on 8 trn2 NeuronCores.

Sharding: core = b*4 + g  (b = batch index, g = head-group of 4 heads).
Each core computes its 4 heads' attention for its batch plus a partial
output projection; the host sums the 4 partials per batch.

RoPE trick: rotate_half(q) = q @ R is linear, so the host precomputes
W_rot = W @ R and the kernel projects twice; q' = (x@W)*cos + (x@W_rot)*sin
is then 3 wide DVE ops in the natural head-contiguous layout:
  qT tile tau rows [64l : 64l+64) = head 2tau+l dims 0..63, T on free.
Scores are computed transposed (S^T = K Q^T, k on partitions, K=64
contraction) so softmax exp feeds the attention matmul with no
transposes; the two heads of a tile sit on disjoint PE row-groups and
psum banks so their score matmuls run concurrently.  V carries an
appended ones column so row 64 of the attention psum is the softmax
denominator.  All matmul operands bf16 (fp32 is 1/4 rate on PE); fp32
accumulation; exp skips fully-invalid diagonal columns.
"""

import numpy as np
import ml_dtypes

BF16 = ml_dtypes.bfloat16

B, T, E = 2, 2048, 1024
H, HD = 16, 64
G = 4             # head groups (cores per batch)
HL = H // G       # heads per core
DL = HL * HD      # local qkv dim = 256
TC = 512          # T chunk (matmul moving free dim)
NJ = T // TC      # 4 q-windows
KC = 128          # k-chunk (contraction tile for attention)
NC_ = T // KC     # 16 k-chunks
SCALE = 1.0 / float(np.sqrt(HD))

_CACHE = {}


def _build_bass():
    import concourse.mybir as mybir
    import concourse.tile as tile
    from concourse import bacc

    f32 = mybir.dt.float32
    bf16 = mybir.dt.bfloat16
    EXP = mybir.ActivationFunctionType.Exp

    nc = bacc.Bacc("TRN2", target_bir_lowering=False, debug=False)
    xt_d = nc.dram_tensor("xt", [E, T], bf16, kind="ExternalInput").ap()
    w_d = nc.dram_tensor("w", [E, 5 * DL], bf16, kind="ExternalInput").ap()
    wo_d = nc.dram_tensor("wo", [DL, E], bf16, kind="ExternalInput").ap()
    cos_d = nc.dram_tensor("cosf", [128, T], bf16, kind="ExternalInput").ap()
    sin_d = nc.dram_tensor("sinf", [128, T], bf16, kind="ExternalInput").ap()
    y_d = nc.dram_tensor("y", [T, E], bf16, kind="ExternalOutput").ap()

    NKK = E // KC  # 8 contraction chunks for the projections

    with tile.TileContext(nc) as tc:
        with (
            tc.tile_pool(name="consts", bufs=1) as consts,
            tc.tile_pool(name="stp", bufs=2, space="PSUM") as stp,
            tc.tile_pool(name="avp", bufs=1, space="PSUM") as avp,
            tc.tile_pool(name="tmp_sb", bufs=6) as tmp_sb,
            tc.tile_pool(name="est_sb", bufs=3) as est_sb,
            tc.tile_pool(name="attn_sb", bufs=1) as attn_sb,
            tc.tile_pool(name="ysb_p", bufs=2) as ysb_p,
            tc.tile_pool(name="small_sb", bufs=2) as small_sb,
        ):
            # ---- constants ----
            xt, w = [], []
            for i in range(NKK):
                tw = consts.tile([KC, 5 * DL], bf16, tag=f"w{i}", name=f"w{i}")
                nc.sync.dma_start(out=tw, in_=w_d[i * KC:(i + 1) * KC, :])
                w.append(tw)
                t = consts.tile([KC, T], bf16, tag=f"xt{i}", name=f"xt{i}")
                nc.sync.dma_start(out=t, in_=xt_d[i * KC:(i + 1) * KC, :])
                xt.append(t)
            wo = []
            for tau in range(2):
                t = consts.tile([128, E], bf16, tag=f"wo{tau}", name=f"wo{tau}")
                nc.sync.dma_start(out=t, in_=wo_d[tau * 128:(tau + 1) * 128, :])
                wo.append(t)
            cosf = consts.tile([128, T], bf16, tag="cosf")
            nc.sync.dma_start(out=cosf, in_=cos_d)
            sinf = consts.tile([128, T], bf16, tag="sinf")
            nc.sync.dma_start(out=sinf, in_=sin_d)

            # triangular band mask: band[p, f] = 1 if f >= p else 0
            band = consts.tile([128, KC], bf16, tag="band")
            nc.gpsimd.memset(band, 1.0)
            nc.gpsimd.affine_select(
                out=band, in_=band, compare_op=mybir.AluOpType.is_ge, fill=0.0,
                base=0, pattern=[[1, KC]], channel_multiplier=-1,
            )

            # resident projection outputs (natural head-contiguous layout)
            qn = [[consts.tile([128, TC], bf16, tag=f"qn{tau}_{j}",
                               name=f"qn{tau}_{j}") for j in range(NJ)]
                  for tau in range(2)]
            kn = [[consts.tile([128, TC], bf16, tag=f"kn{tau}_{j}",
                               name=f"kn{tau}_{j}") for j in range(NJ)]
                  for tau in range(2)]
            vsb = [consts.tile([128, HL * 65], bf16, tag=f"v{c}", name=f"v{c}")
                   for c in range(NC_)]

            def emit_proj_qk(jj, base, tau, dst):
                """Project [plain | rotated] m-chunks for head pair tau of
                window jj, then RoPE-combine into dst[tau][jj]."""
                js = slice(jj * TC, (jj + 1) * TC)
                ps = stp.tile([128, 2 * TC], f32, tag="st",
                              name=f"pqk{base}_{tau}_{jj}")
                for r in range(2):
                    cc = base + DL * r + 128 * tau
                    for kk in range(NKK):
                        nc.tensor.matmul(
                            ps[:, r * TC:(r + 1) * TC],
                            lhsT=w[kk][:, cc:cc + 128],
                            rhs=xt[kk][:, js],
                            start=(kk == 0), stop=(kk == NKK - 1))
                ta = tmp_sb.tile([128, TC], f32, tag="ropeA",
                                 name=f"ra{base}_{tau}_{jj}")
                tb = tmp_sb.tile([128, TC], f32, tag="ropeB",
                                 name=f"rb{base}_{tau}_{jj}")
                nc.vector.tensor_mul(ta, ps[:, 0:TC], cosf[:, js])
                nc.vector.tensor_mul(tb, ps[:, TC:2 * TC], sinf[:, js])
                nc.vector.tensor_add(dst[tau][jj], ta, tb)

            def emit_proj_v(jj):
                js0 = jj * (TC // KC)
                ps = stp.tile([128, 4 * DL], f32, tag="st", name=f"pv{jj}")
                for tt in range(TC // KC):
                    c = js0 + tt
                    for kk in range(NKK):
                        nc.tensor.matmul(
                            ps[:, tt * DL:(tt + 1) * DL],
                            lhsT=xt[kk][:, c * KC:(c + 1) * KC],
                            rhs=w[kk][:, 4 * DL:5 * DL],
                            start=(kk == 0), stop=(kk == NKK - 1))
                for tt in range(TC // KC):
                    c = js0 + tt
                    vv = vsb[c].rearrange("p (h d) -> p h d", h=HL)
                    nc.gpsimd.memset(vv[:, :, 64:65], 1.0)
                    nc.vector.tensor_copy(
                        vv[:, :, 0:64],
                        ps[:, tt * DL:(tt + 1) * DL].rearrange(
                            "p (h d) -> p h d", h=HL))

            def emit_proj(jj, part):
                if part == 0:
                    emit_proj_qk(jj, 0, 0, qn)
                elif part == 1:
                    emit_proj_qk(jj, 0, 1, qn)
                elif part == 2:
                    emit_proj_qk(jj, 2 * DL, 0, kn)
                elif part == 3:
                    emit_proj_qk(jj, 2 * DL, 1, kn)
                else:
                    emit_proj_v(jj)

            def emit_y(jj, part):
                n, th = divmod(part, 2)
                at = ats[jj]
                yp = stp.tile([128, 2 * TC], f32, tag="st",
                              name=f"yp{jj}_{n}_{th}")
                for ti in range(2):
                    tt = 2 * th + ti
                    for tau in range(2):
                        nc.tensor.matmul(
                            yp[:, ti * TC:(ti + 1) * TC],
                            lhsT=at[tau][:, tt * KC:(tt + 1) * KC],
                            rhs=wo[tau][:, n * TC:(n + 1) * TC],
                            start=(tau == 0), stop=(tau == 1))
                ys = ysb_p.tile([128, 2 * TC], bf16, tag="y",
                                name=f"ys{jj}_{n}_{th}")
                nc.vector.tensor_copy(ys, yp)
                nc.sync.dma_start(
                    out=y_d[jj * TC + th * 2 * KC:
                            jj * TC + (th + 1) * 2 * KC,
                            n * TC:(n + 1) * TC].rearrange(
                                "(a p) c -> p a c", p=128),
                    in_=ys.rearrange("p (a c) -> p a c", a=2))

            # prologue: projections for window 0
            for part in range(5):
                emit_proj(0, part)
            ats = []

            for j in range(NJ):
                nch = 4 * (j + 1)          # causal k-chunks for this window
                # interleave next window's projection emission into the
                # attention chunk loop (keeps PE busy while ACT does exp)
                proj_at = {}
                if j + 1 < NJ:
                    for part in range(5):
                        cpos = max(0, (nch * (part + 1)) // 6 - 1)
                        proj_at.setdefault(cpos, []).append(("p", part))


                at = [attn_sb.tile([128, TC], bf16, tag=f"attn{tau}_{j}",
                                   name=f"attn{tau}_{j}") for tau in range(2)]
                ats.append(at)
                av4 = avp.tile([128, 4 * TC], f32, tag="av", name=f"av_{j}")
                for c in range(nch):
                    d = c - 4 * j          # 0..3 on the diagonal
                    sts = [stp.tile([128, 2 * TC], f32, tag="st",
                                    name=f"st{j}_{c}_{i}") for i in range(2)]
                    # one K=64 matmul per head; the two heads of a tile sit
                    # on disjoint row-groups and psum banks
                    for ll in range(2):
                        for tau in range(2):
                            h = 2 * tau + ll
                            stt = sts[h // 2]
                            wnd = slice((h % 2) * TC, (h % 2 + 1) * TC)
                            nc.tensor.matmul(
                                stt[:, wnd],
                                lhsT=kn[tau][c // 4][
                                    64 * ll:64 * ll + 64,
                                    (c % 4) * KC:(c % 4 + 1) * KC],
                                rhs=qn[tau][j][64 * ll:64 * ll + 64, :],
                                start=True, stop=True,
                                tile_position=(64 * ll, 0))
                    ests = []
                    ecoff = KC * d if d > 0 else 0
                    for i in range(2):
                        est = est_sb.tile([128, 2 * TC], bf16, tag="est",
                                          name=f"est{j}_{c}_{i}")
                        if ecoff:
                            nc.scalar.activation(
                                est.rearrange("p (w c) -> p w c", w=2)[
                                    :, :, ecoff:],
                                sts[i].rearrange("p (w c) -> p w c", w=2)[
                                    :, :, ecoff:],
                                EXP, scale=SCALE)
                        else:
                            nc.scalar.activation(est, sts[i], EXP, scale=SCALE)
                        ests.append(est)
                    if d >= 0:
                        for h in range(HL):
                            bs = slice((h % 2) * TC + KC * d,
                                       (h % 2) * TC + KC * (d + 1))
                            nc.vector.tensor_mul(
                                ests[h // 2][:, bs], ests[h // 2][:, bs], band)
                    coff = KC * d if d > 0 else 0
                    for h in range(HL):
                        nc.tensor.matmul(
                            av4[0:65, h * TC + coff:(h + 1) * TC],
                            lhsT=vsb[c][:, 65 * h:65 * h + 65],
                            rhs=ests[h // 2][:, (h % 2) * TC + coff:
                                             (h % 2 + 1) * TC],
                            start=(c == 0), stop=(c == nch - 1))
                    for kind, part in proj_at.get(c, ()):
                        if kind == "p":
                            emit_proj(j + 1, part)
                        else:
                            emit_y(j - 1, part)

                # stage raw attention psum to SBUF (frees the psum slot
                # for the next window), then divide by the denominator
                avv = tmp_sb.tile([64, 4 * TC], f32, tag="avstage",
                                  name=f"avv{j}")
                nc.vector.tensor_copy(avv, av4[0:64, :])
                avd = small_sb.tile([1, 4 * TC], f32, tag="denom",
                                    name=f"avd{j}")
                nc.vector.tensor_copy(avd, av4[64:65, :])
                for h in range(HL):
                    hw = slice(h * TC, (h + 1) * TC)
                    rc = small_sb.tile([1, TC], f32, tag="recip",
                                       name=f"rc{j}_{h}")
                    nc.vector.reciprocal_approx_fast(out=rc, in_=avd[:, hw])
                    rb = small_sb.tile([64, TC], f32, tag="rbcast",
                                       name=f"rb{j}_{h}")
                    nc.gpsimd.partition_broadcast(rb, rc)
                    nc.vector.tensor_mul(
                        at[h // 2][64 * (h % 2):64 * (h % 2) + 64, :],
                        avv[:, hw], rb)

            # ---- epilogue: output projection for all windows ----
            for jj in range(NJ):
                for part in range(4):
                    emit_y(jj, part)

    nc.compile()
    return nc


def _host_inputs(x, cos, sin, w_qkv, w_out):
    """Shard + lay out the full inputs for the 8 cores."""
    # natural-layout tables: row 64*l + d = cos/sin[t, d]
    cosf = np.ascontiguousarray(np.tile(cos.T, (2, 1))).astype(BF16)
    sinf = np.ascontiguousarray(np.tile(sin.T, (2, 1))).astype(BF16)

    xts = [np.ascontiguousarray(x[b].T).astype(BF16) for b in range(B)]

    # rotate_half as a column transform: per head, W_rot[:, d] = -W[:, d+32]
    # for d<32 and W_rot[:, 32+d] = W[:, d]
    def rot_cols(wblk):
        out = np.empty_like(wblk)
        for h in range(HL):
            hs = wblk[:, h * HD:(h + 1) * HD]
            o = out[:, h * HD:(h + 1) * HD]
            o[:, 0:32] = -hs[:, 32:64]
            o[:, 32:64] = hs[:, 0:32]
        return out

    in_maps = []
    for core in range(8):
        b, g = divmod(core, G)
        qblk = w_qkv[:, G * g * HD:(G * g + HL) * HD]
        kblk = w_qkv[:, E + G * g * HD:E + (G * g + HL) * HD]
        vblk = w_qkv[:, 2 * E + DL * g:2 * E + DL * (g + 1)]
        wl = np.concatenate(
            [qblk, rot_cols(qblk), kblk, rot_cols(kblk), vblk],
            axis=1).astype(BF16)                                   # (E, 1280)
        wol = np.ascontiguousarray(w_out[DL * g:DL * (g + 1), :]).astype(BF16)
        in_maps.append({
            "xt": xts[b], "w": wl, "wo": wol, "cosf": cosf, "sinf": sinf,
        })
    return in_maps


def kernel(x, cos, sin, w_qkv, w_out):
    from concourse import bass_utils

    if "nc" not in _CACHE:
        _CACHE["nc"] = _build_bass()
    nc = _CACHE["nc"]

    in_maps = _host_inputs(
        np.asarray(x, dtype=np.float32), np.asarray(cos, dtype=np.float32),
        np.asarray(sin, dtype=np.float32), np.asarray(w_qkv, dtype=np.float32),
        np.asarray(w_out, dtype=np.float32))

    res = bass_utils.run_bass_kernel_spmd(nc, in_maps, core_ids=list(range(8)))

    y = np.zeros((B, T, E), dtype=np.float32)
    for core in range(8):
        b = core // G
        y[b] += res.results[core]["y"].astype(np.float32)
    return y
